# revision 7
# baseline (speedup 1.0000x reference)
"""CausalGNN forward on 8 Trainium2 NeuronCores (Bass/Tile) — v4.

Math (PyG-style GCN, 3 layers, BN training-mode, residuals):
    deg[v] = 1 + #{edges with dst=v};  dis = deg^-1/2
    per layer i:  agg[v] = sum_{e=(u,v)} dis_u dis_v x[u] + dis_v^2 x[v]
                  z = W_i^T agg   (aggregate-first: A-hat and W commute)
                  y = BN(z) (bias b_i cancels in BN), ReLU if i<2
                  x = y (i=0) or x + y (i>0)

v4 restructuring (vs v3):
  - edge norm dis_u*dis_v is host-computed per edge slot and folded into the
    one-hot VALUES (one-hot[e, dst_e] = norm_e), so the gather table holds
    RAW values — no dis scaling on device at all.
  - the gather table for layer i+1 IS the z AllGather output (node-major
    bf16): no per-core full-table rebuild. Gathered z is transformed in
    place per strip: r = relu(z + c_i), c_i = bn_bias/bn_scale (bn_scale>0),
    and the bn_scale is folded into the next layer's weights
    W' = diag(s) @ W.
  - the layer-2 residual (x2 = s0*r0 + s1*r1) reuses the SAVED layer-1
    aggregate A1 = sum norm*r0:  z2 = (diag(s1)W2)^T A2 + (diag(s0)W2)^T A1
    — two accumulating matmuls, no second gather.
  - BN partial sums ride the z AllGather as 4 bf16-bitcast rows appended to
    each core's slab (pure DMA moves, bit-preserving). The layer-2 BN is
    applied on the HOST during output assembly (the device returns x2 and
    raw z2; BN2 is a pointwise affine with globally-exact stats computed in
    numpy) — 2 collectives total.
  - edges aggregated via batched nc.gpsimd.dma_gather (1024 idxs/call; more
    overflows the 16KB SWDGE descriptor ring), round-robined over 4 SWDGE
    queues, with slots sorted by source row inside each segment (ascending
    HBM addresses). Aggregation one-hot tiles (value = norm_e) are
    precomputed on host and streamed from DRAM; PSUM-accumulated per
    128-node chunk.
  - host prep is pure graph-structure bookkeeping on edge_index (bucket/
    sort/pad + degree counts + per-slot norm weights); everything touching
    x flows through the device.
"""
import sys
sys.path.insert(0, "/opt/trn_rl_repo")

import numpy as np
import ml_dtypes

import concourse.bass as bass
import concourse.tile as tile
from concourse import bacc, mybir

f32 = mybir.dt.float32
bf16 = mybir.dt.bfloat16
i16 = mybir.dt.int16
i32 = mybir.dt.int32

P = 128
CORES = 8
L = 3
EPS = 1e-5
HALF = 32768
GRP = 4
NSTRIP = 8
SPAD = 4          # stats rows appended to each core's table slab
SORT_SROW = True  # ascending-srow slot order within segments (gather locality)


# ---------------------------------------------------------------- host prep

def _prep(x, edge_index):
    """Bucket edges by (core, chunk, table-half); append self-edges; pad each
    segment to 128-edge tiles on a schedule shared by all cores. Compute the
    per-slot aggregation weight norm_e = dis_src*dis_dst (self: dis^2).

    Returns per-core idx/rel/norm planes, the shared group schedule, and the
    node-major bf16 input table."""
    N, D = x.shape
    n_own = (N + CORES - 1) // CORES
    n_pad = ((n_own + P - 1) // P) * P
    n_chunks = n_pad // P
    SLAB = n_pad + SPAD
    TBL = CORES * SLAB

    src = edge_index[0].astype(np.int64)
    dst = edge_index[1].astype(np.int64)

    deg = np.bincount(dst, minlength=N).astype(np.float64) + 1.0
    dis = 1.0 / np.sqrt(deg)

    # real edges
    core_r = dst // n_own
    local_r = dst - core_r * n_own
    chunk_r = local_r // P
    rel_r = local_r % P
    srow_r = (src // n_own) * SLAB + (src - (src // n_own) * n_own)
    norm_r = (dis[src] * dis[dst]).astype(np.float32)

    # self edges: every (core, chunk, col) slot; pad slots point at own row 0
    cc, chch, rr = np.meshgrid(np.arange(CORES), np.arange(n_chunks),
                               np.arange(P), indexing="ij")
    cc, chch, rr = cc.ravel(), chch.ravel(), rr.ravel()
    nl = chch * P + rr
    is_real = (nl < n_own) & (cc * n_own + nl < N)
    srow_s = cc * SLAB + np.where(is_real, nl, 0)
    gid = np.where(is_real, cc * n_own + nl, 0)
    norm_s = np.where(is_real, (dis * dis)[gid], 0.0).astype(np.float32)

    core_a = np.concatenate([cc, core_r])
    chunk_a = np.concatenate([chch, chunk_r])
    rel_a = np.concatenate([rr, rel_r])
    srow_a = np.concatenate([srow_s, srow_r])
    norm_a = np.concatenate([norm_s, norm_r])
    half_a = (srow_a >= HALF).astype(np.int64)

    # shared tile schedule: kA/kB per chunk = max over cores
    counts = np.zeros((CORES, n_chunks, 2), np.int64)
    np.add.at(counts, (core_a, chunk_a, half_a), 1)
    kA = -(-counts[:, :, 0].max(axis=0) // P)      # [n_chunks]
    kB = -(-counts[:, :, 1].max(axis=0) // P)

    groups = []          # (chunk_list, kA_list, kB_list, tile_base)
    tb = 0
    for g0 in range(0, n_chunks, GRP):
        chs = list(range(g0, min(g0 + GRP, n_chunks)))
        groups.append((chs, [int(kA[c]) for c in chs],
                       [int(kB[c]) for c in chs], tb))
        tb += int(kA[chs[0]:chs[-1] + 1].sum() + kB[chs[0]:chs[-1] + 1].sum())
    NT = tb

    # global tile base per (chunk, half), following group order:
    # group layout: [A tiles chunk-major][B tiles chunk-major]
    tile_base = np.zeros((n_chunks, 2), np.int64)
    for chs, kAl, kBl, base in groups:
        off = base
        for c, k in zip(chs, kAl):
            tile_base[c, 0] = off
            off += k
        for c, k in zip(chs, kBl):
            tile_base[c, 1] = off
            off += k

    idx_arr = np.zeros((CORES, 16, 8 * NT), np.int16)
    rel_arr = np.full((CORES, P, NT), -1.0, np.float32)
    norm_arr = np.zeros((CORES, P, NT), np.float32)

    key = (core_a * n_chunks + chunk_a) * 2 + half_a
    # within each (core, chunk, half) segment, order slots by source row:
    # the gather then reads HBM in ascending address order (slot order
    # within a tile is free — rel/norm ride along).
    order = (np.lexsort((srow_a, key)) if SORT_SROW
             else np.argsort(key, kind="stable"))
    key_s = key[order]
    starts = np.searchsorted(key_s, np.arange(CORES * n_chunks * 2))
    pos = np.arange(key.size) - starts[key_s]
    tg = tile_base[chunk_a[order], half_a[order]] + pos // P
    q = pos % P
    col = tg * 8 + q // 16
    prow = q % 16
    co = core_a[order]
    idx_arr[co, prow, col] = (srow_a[order] - half_a[order] * HALF).astype(
        np.int16)
    rel_arr[co, q, tg] = rel_a[order]
    norm_arr[co, q, tg] = norm_a[order]

    idx_arr = np.tile(idx_arr, (1, 8, 1))  # replicate across gpsimd cores

    # replicated full x, node-major bf16, padded to table-slab layout
    xfull = np.zeros((TBL, D), ml_dtypes.bfloat16)
    for c in range(CORES):
        lo, hi = c * n_own, min((c + 1) * n_own, N)
        xfull[c * SLAB:c * SLAB + hi - lo] = x[lo:hi].astype(
            ml_dtypes.bfloat16)

    return (xfull, idx_arr, rel_arr, norm_arr, groups, NT, n_own, n_pad,
            n_chunks)


# ------------------------------------------------------------- device build

def _build(D, n_pad, n_chunks, NT, groups, n_real_last, N_total):
    nc = bacc.Bacc("TRN2", target_bir_lowering=False, debug=False,
                   num_devices=CORES, num_swdge_queues=4)
    SLAB = n_pad + SPAD
    TBL = CORES * SLAB

    xfull_in = nc.dram_tensor("xfull_in", [TBL, D], bf16,
                              kind="ExternalInput")
    idx_in = nc.dram_tensor("idx_in", [P, 8 * NT], i16, kind="ExternalInput")
    rel_in = nc.dram_tensor("rel_in", [P, NT], f32, kind="ExternalInput")
    norm_in = nc.dram_tensor("norm_in", [P, NT], f32, kind="ExternalInput")
    Ws_in = nc.dram_tensor("Ws_in", [L * D, D], f32, kind="ExternalInput")
    gb_in = nc.dram_tensor("gb_in", [D, 2 * L], f32, kind="ExternalInput")
    out_ext = nc.dram_tensor("out", [D, n_pad], f32, kind="ExternalOutput")
    outz_ext = nc.dram_tensor("outz", [D, n_pad], f32, kind="ExternalOutput")

    zst = nc.dram_tensor("zst", [SLAB, D], bf16)
    zg = [nc.dram_tensor(f"zg{i}", [TBL, D], bf16, addr_space="Shared")
          for i in range(L - 1)]

    RG = [list(range(CORES))]
    AOP = mybir.AluOpType
    ACT = mybir.ActivationFunctionType

    SA = max(-(-sum(kAl) // NSTRIP) for _, kAl, _, _ in groups)
    SB = max(max(-(-sum(kBl) // NSTRIP) for _, _, kBl, _ in groups), 1)

    with tile.TileContext(nc) as tc:
        with tc.tile_pool(name="big", bufs=1) as big, \
             tc.tile_pool(name="sm", bufs=1) as sm, \
             tc.tile_pool(name="gatA", bufs=2 * SA + 1) as gatA, \
             tc.tile_pool(name="gatB", bufs=2 * SB + 1) as gatB, \
             tc.tile_pool(name="oh", bufs=6) as ohp, \
             tc.tile_pool(name="work", bufs=3) as wk, \
             tc.tile_pool(name="ps", bufs=2, space="PSUM") as ps, \
             tc.tile_pool(name="psz", bufs=2, space="PSUM") as psz, \
             tc.tile_pool(name="pst", bufs=2, space="PSUM") as pst, \
             tc.tile_pool(name="psd", bufs=1, space="PSUM") as psd:

            # ---------------- persistent SBUF state
            idx_sb = big.tile([P, 8 * NT], i16)
            nc.sync.dma_start(out=idx_sb[:], in_=idx_in[:, :])
            rel_sb = big.tile([P, NT], f32)
            nc.sync.dma_start(out=rel_sb[:], in_=rel_in[:, :])
            norm_sb = big.tile([P, NT], f32)
            nc.sync.dma_start(out=norm_sb[:], in_=norm_in[:, :])
            gb_sb = sm.tile([D, 2 * L], f32)
            nc.sync.dma_start(out=gb_sb[:], in_=gb_in[:, :])
            Ws_f32 = sm.tile([D, L * D], f32)
            for i in range(L):
                nc.sync.dma_start(out=Ws_f32[:, i * D:(i + 1) * D],
                                  in_=Ws_in[i * D:(i + 1) * D, :])
            # effective bf16 weights: [W0 | s0*W1 | s1*W2 | s0*W2]
            Wb = sm.tile([D, 4 * D], bf16)
            nc.scalar.copy(out=Wb[:, 0:D], in_=Ws_f32[:, 0:D])

            iota_i = sm.tile([P, P], i32)
            nc.gpsimd.iota(iota_i[:], pattern=[[1, P]], base=0,
                           channel_multiplier=0)
            iota_f = sm.tile([P, P], f32)
            nc.vector.tensor_copy(iota_f[:], iota_i[:])
            iota_bf = sm.tile([P, P], bf16)
            nc.vector.tensor_copy(iota_bf[:], iota_i[:])
            iota_col_i = sm.tile([P, P], i32)
            nc.gpsimd.iota(iota_col_i[:], pattern=[[1, P]], base=0,
                           channel_multiplier=1)
            iota_col = sm.tile([P, 1], f32)
            nc.vector.tensor_copy(iota_col[:], iota_col_i[:, 0:1])
            ident_bf = sm.tile([P, P], bf16)
            nc.vector.tensor_scalar(out=ident_bf[:], in0=iota_f[:],
                                    scalar1=iota_col[:], scalar2=None,
                                    op0=AOP.is_equal)
            ones_bsq = sm.tile([P, P], bf16)
            nc.vector.memset(ones_bsq[:], 1.0)

            xT = big.tile([D, n_pad], f32)       # residual master
            zbuf = big.tile([D, n_pad], f32)     # pre-BN z per layer
            A1 = big.tile([D, n_pad], bf16)      # saved layer-1 aggregate
            n_own_cols = (n_chunks - 1) * P + n_real_last
            if n_own_cols < n_pad:
                nc.vector.memset(zbuf[:, n_own_cols:], 0.0)
            slots = sm.tile([P, 2 * n_chunks], f32)
            stat = sm.tile([P, 12], f32)
            scol = sm.tile([P, 2], f32)          # saved s0, s1
            crep = sm.tile([P, NSTRIP * P], bf16)

            inv_n = 1.0 / float(N_total)
            for i in range(L):
                # ---- per-layer setup from previous layer's BN stats
                if i > 0:
                    # crep = broadcast of c_{i-1} = bias/scale across rows
                    diag_c = wk.tile([P, P], bf16, tag="diagc")
                    nc.vector.tensor_scalar(
                        out=diag_c[:], in0=iota_f[:], scalar1=iota_col[:],
                        scalar2=stat[:, 8:9], op0=AOP.is_equal, op1=AOP.mult)
                    bps = psd.tile([P, P], f32, space="PSUM", tag="bc")
                    nc.tensor.matmul(out=bps[:], lhsT=ones_bsq[:],
                                     rhs=diag_c[:], start=True, stop=True)
                    nc.vector.tensor_copy(crep[:, 0:P], bps[:])
                    for r in range(1, NSTRIP):
                        nc.vector.tensor_copy(crep[:, r * P:(r + 1) * P],
                                              crep[:, 0:P])
                    # effective weights for this layer
                    wtmp = wk.tile([D, P], f32, tag="wtmp")
                    nc.vector.tensor_scalar(
                        out=wtmp[:], in0=Ws_f32[:, i * D:(i + 1) * D],
                        scalar1=scol[:, i - 1:i], scalar2=None, op0=AOP.mult)
                    nc.scalar.copy(out=Wb[:, i * D:(i + 1) * D], in_=wtmp[:])
                    if i == 2:
                        nc.vector.tensor_scalar(
                            out=wtmp[:], in0=Ws_f32[:, 2 * D:3 * D],
                            scalar1=scol[:, 0:1], scalar2=None, op0=AOP.mult)
                        nc.scalar.copy(out=Wb[:, 3 * D:4 * D], in_=wtmp[:])

                src_t = xfull_in if i == 0 else zg[i - 1]

                # ---- edge phase
                qctr = 0
                for chs, kAl, kBl, base in groups:
                    KA, KB = sum(kAl), sum(kBl)
                    cA = 8 * base
                    cB = 8 * (base + KA)
                    # gathers in strips of <=NSTRIP tiles (1024 idxs): a
                    # single SWDGE call beyond ~1024 descriptors overflows
                    # the 16KB DynamicDMAScratch ring and wedges the device.
                    gAs = []
                    for s0 in range(0, KA, NSTRIP):
                        n = min(NSTRIP, KA - s0)
                        gs = gatA.tile([P, NSTRIP * P], bf16, tag="gA")
                        nc.gpsimd.dma_gather(
                            out_ap=gs[:, :n * P].rearrange(
                                "p (k e) -> p k e", e=P),
                            in_ap=src_t[0:HALF, :],
                            idxs_ap=idx_sb[:, cA + 8 * s0:cA + 8 * (s0 + n)],
                            num_idxs=n * P, num_idxs_reg=n * P, elem_size=P,
                            queue_num=qctr % 4)
                        qctr += 1
                        if i > 0:
                            nc.vector.tensor_tensor(
                                out=gs[:, :n * P], in0=gs[:, :n * P],
                                in1=crep[:, :n * P], op=AOP.add)
                            nc.scalar.activation(
                                out=gs[:, :n * P], in_=gs[:, :n * P],
                                func=ACT.Relu)
                        gAs.append(gs)
                    gBs = []
                    for s0 in range(0, KB, NSTRIP):
                        n = min(NSTRIP, KB - s0)
                        gs = gatB.tile([P, NSTRIP * P], bf16, tag="gB")
                        nc.gpsimd.dma_gather(
                            out_ap=gs[:, :n * P].rearrange(
                                "p (k e) -> p k e", e=P),
                            in_ap=src_t[HALF:TBL, :],
                            idxs_ap=idx_sb[:, cB + 8 * s0:cB + 8 * (s0 + n)],
                            num_idxs=n * P, num_idxs_reg=n * P, elem_size=P,
                            queue_num=qctr % 4)
                        qctr += 1
                        if i > 0:
                            nc.vector.tensor_tensor(
                                out=gs[:, :n * P], in0=gs[:, :n * P],
                                in1=crep[:, :n * P], op=AOP.add)
                            nc.scalar.activation(
                                out=gs[:, :n * P], in_=gs[:, :n * P],
                                func=ACT.Relu)
                        gBs.append(gs)
                    offA = 0
                    offB = 0
                    tA = base
                    tB = base + KA
                    for ci, c in enumerate(chs):
                        aps = ps.tile([D, P], f32, space="PSUM", tag="agg")
                        ntile = kAl[ci] + kBl[ci]
                        n = 0
                        for j in range(kAl[ci]):
                            oht = ohp.tile([P, P], bf16, tag="oh")
                            nc.vector.tensor_scalar(
                                out=oht[:], in0=iota_f[:],
                                scalar1=rel_sb[:, tA + j:tA + j + 1],
                                scalar2=norm_sb[:, tA + j:tA + j + 1],
                                op0=AOP.is_equal, op1=AOP.mult)
                            ja = offA + j
                            nc.tensor.matmul(
                                out=aps[:],
                                lhsT=gAs[ja // NSTRIP][
                                    :, (ja % NSTRIP) * P:(ja % NSTRIP + 1) * P],
                                rhs=oht[:], start=(n == 0),
                                stop=(n == ntile - 1))
                            n += 1
                        for j in range(kBl[ci]):
                            oht = ohp.tile([P, P], bf16, tag="oh")
                            nc.vector.tensor_scalar(
                                out=oht[:], in0=iota_f[:],
                                scalar1=rel_sb[:, tB + j:tB + j + 1],
                                scalar2=norm_sb[:, tB + j:tB + j + 1],
                                op0=AOP.is_equal, op1=AOP.mult)
                            jb = offB + j
                            nc.tensor.matmul(
                                out=aps[:],
                                lhsT=gBs[jb // NSTRIP][
                                    :, (jb % NSTRIP) * P:(jb % NSTRIP + 1) * P],
                                rhs=oht[:], start=(n == 0),
                                stop=(n == ntile - 1))
                            n += 1
                        offA += kAl[ci]
                        offB += kBl[ci]
                        tA += kAl[ci]
                        tB += kBl[ci]
                        # agg -> bf16; z = W'^T agg (+ W''^T A1 at layer 2)
                        if i == 1:
                            aggb = A1[:, c * P:(c + 1) * P]
                            nc.vector.tensor_copy(aggb, aps[:])
                        else:
                            aggt = wk.tile([D, P], bf16, tag="aggS")
                            nc.vector.tensor_copy(aggt[:], aps[:])
                            aggb = aggt[:]
                        zps = psz.tile([D, P], f32, space="PSUM", tag="z")
                        if i < 2:
                            nc.tensor.matmul(out=zps[:],
                                             lhsT=Wb[:, i * D:(i + 1) * D],
                                             rhs=aggb, start=True, stop=True)
                        else:
                            nc.tensor.matmul(out=zps[:],
                                             lhsT=Wb[:, 2 * D:3 * D],
                                             rhs=aggb, start=True, stop=False)
                            nc.tensor.matmul(out=zps[:],
                                             lhsT=Wb[:, 3 * D:4 * D],
                                             rhs=A1[:, c * P:(c + 1) * P],
                                             start=False, stop=True)
                        # copy z -> zbuf, fused BN partial sums via accum_out
                        w = P if c < n_chunks - 1 else n_real_last
                        s = slice(c * P, c * P + w)
                        if i < 2:
                            nc.scalar.activation(
                                out=zbuf[:, s], in_=zps[:, 0:w],
                                func=ACT.Identity,
                                accum_out=slots[:, c:c + 1])
                            sqs = wk.tile([D, P], bf16, tag="sq")
                            nc.scalar.activation(
                                out=sqs[:, 0:w], in_=zps[:, 0:w],
                                func=ACT.Square,
                                accum_out=slots[:, n_chunks + c:n_chunks + c + 1])
                        else:
                            nc.vector.tensor_copy(zbuf[:, s], zps[:, 0:w])
                        if i < 2:
                            # node-major bf16 z into the AllGather slab
                            zc = wk.tile([D, P], bf16, tag="zc")
                            nc.vector.tensor_copy(zc[:], zps[:])
                            tps = pst.tile([P, P], bf16, space="PSUM",
                                           tag="tp")
                            nc.tensor.transpose(tps[:], zc[:], ident_bf[:])
                            zrow = wk.tile([P, P], bf16, tag="zrow")
                            nc.vector.tensor_copy(zrow[:], tps[:])
                            nc.sync.dma_start(
                                out=zst[c * P:(c + 1) * P, :], in_=zrow[:])

                if i == L - 1:
                    # raw z2 + x2 go to the host; BN2 applied there
                    nc.sync.dma_start(out=outz_ext[:, :], in_=zbuf[:])
                    break
                # ---- BN stats: reduce chunk slots; the partials ride the
                # z AllGather (bitcast into 4 bf16 rows, bit-preserving DMA
                # only); every core sums the 8 partials.
                nc.vector.tensor_reduce(
                    out=stat[:, 0:1], in_=slots[:, 0:n_chunks],
                    axis=mybir.AxisListType.X, op=AOP.add)
                nc.vector.tensor_reduce(
                    out=stat[:, 1:2], in_=slots[:, n_chunks:2 * n_chunks],
                    axis=mybir.AxisListType.X, op=AOP.add)
                sin = wk.tile([P, 2], f32, tag="stin")
                nc.vector.tensor_copy(sin[:], stat[:, 0:2])
                sout = wk.tile([P, 2], f32, tag="stout")
                nc.sync.dma_start(
                    out=zst[n_pad:n_pad + SPAD, :].rearrange("r d -> d r"),
                    in_=sin[:].bitcast(bf16))
                nc.gpsimd.collective_compute(
                    "AllGather", AOP.bypass, replica_groups=RG,
                    ins=[zst[:, :]], outs=[zg[i][:, :]])
                srd = wk.tile([P, SPAD * CORES], bf16, tag="srd")
                for cc in range(CORES):
                    r0 = cc * SLAB + n_pad
                    nc.sync.dma_start(
                        out=srd[:, cc * SPAD:(cc + 1) * SPAD],
                        in_=zg[i][r0:r0 + SPAD, :].rearrange("r d -> d r"))
                srdf = srd[:].bitcast(f32)
                nc.vector.tensor_copy(sout[:], srdf[:, 0:2])
                for cc in range(1, CORES):
                    nc.vector.tensor_tensor(
                        out=sout[:], in0=sout[:],
                        in1=srdf[:, 2 * cc:2 * cc + 2], op=AOP.add)
                nc.vector.tensor_scalar(out=stat[:, 2:3], in0=sout[:, 0:1],
                                        scalar1=inv_n, scalar2=None,
                                        op0=AOP.mult)           # mean
                nc.vector.tensor_scalar(out=stat[:, 3:4], in0=sout[:, 1:2],
                                        scalar1=inv_n, scalar2=None,
                                        op0=AOP.mult)           # E[z^2]
                nc.vector.tensor_tensor(out=stat[:, 4:5], in0=stat[:, 2:3],
                                        in1=stat[:, 2:3], op=AOP.mult)
                nc.vector.tensor_tensor(out=stat[:, 4:5], in0=stat[:, 3:4],
                                        in1=stat[:, 4:5], op=AOP.subtract)
                nc.vector.tensor_scalar(out=stat[:, 4:5], in0=stat[:, 4:5],
                                        scalar1=float(EPS), scalar2=None,
                                        op0=AOP.add)            # var+eps
                nc.vector.reciprocal(stat[:, 5:6], stat[:, 4:5])
                nc.scalar.sqrt(stat[:, 6:7], stat[:, 5:6])      # rsqrt
                nc.vector.tensor_tensor(out=stat[:, 6:7],
                                        in0=gb_sb[:, 2 * i:2 * i + 1],
                                        in1=stat[:, 6:7], op=AOP.mult)
                nc.vector.tensor_tensor(out=stat[:, 7:8], in0=stat[:, 6:7],
                                        in1=stat[:, 2:3], op=AOP.mult)
                nc.vector.tensor_tensor(out=stat[:, 7:8],
                                        in0=gb_sb[:, 2 * i + 1:2 * i + 2],
                                        in1=stat[:, 7:8], op=AOP.subtract)
                # stat6 = s (scale), stat7 = b (bias)
                nc.vector.tensor_copy(scol[:, i:i + 1], stat[:, 6:7])
                nc.vector.reciprocal(stat[:, 9:10], stat[:, 6:7])
                nc.vector.tensor_tensor(out=stat[:, 8:9],
                                        in0=stat[:, 7:8],
                                        in1=stat[:, 9:10], op=AOP.mult)

                # ---- y = relu(scale*z + shift); residual (own nodes)
                for c in range(n_chunks):
                    s = slice(c * P, (c + 1) * P)
                    if i == 0:
                        nc.scalar.activation(out=xT[:, s], in_=zbuf[:, s],
                                             func=ACT.Relu, bias=stat[:, 7:8],
                                             scale=stat[:, 6:7])
                    else:
                        yt = wk.tile([D, P], f32, tag="y")
                        nc.scalar.activation(out=yt[:], in_=zbuf[:, s],
                                             func=ACT.Relu, bias=stat[:, 7:8],
                                             scale=stat[:, 6:7])
                        nc.vector.tensor_tensor(out=xT[:, s], in0=xT[:, s],
                                                in1=yt[:], op=AOP.add)

            nc.sync.dma_start(out=out_ext[:, :], in_=xT[:])
    nc.compile()
    return nc


# ------------------------------------------------------------------ runner

class _Runner:
    """Persistent-jit PJRT runner (run_bass_via_pjrt, callable repeatedly)."""

    def __init__(self, nc, n_cores):
        import jax
        from jax.experimental.shard_map import shard_map
        from jax.sharding import Mesh, PartitionSpec
        from concourse import bass2jax
        self.jax = jax
        bass2jax.install_neuronx_cc_hook()
        in_names, out_names, out_avals, zero_outs = [], [], [], []
        partition_name = (nc.partition_id_tensor.name
                          if nc.partition_id_tensor else None)
        for alloc in nc.m.functions[0].allocations:
            if not isinstance(alloc, mybir.MemoryLocationSet):
                continue
            name = alloc.memorylocations[0].name
            if alloc.kind == "ExternalInput":
                if name != partition_name:
                    in_names.append(name)
            elif alloc.kind == "ExternalOutput":
                out_names.append(name)
                shape = tuple(alloc.tensor_shape)
                dtype = mybir.dt.np(alloc.dtype)
                out_avals.append(jax.core.ShapedArray(shape, dtype))
                zero_outs.append(np.zeros(shape, dtype))
        self.in_names, self.out_names = in_names, out_names
        self.out_avals, self.zero_outs = out_avals, zero_outs
        n_params, n_outs = len(in_names), len(out_avals)
        all_in = list(in_names) + list(out_names)
        if partition_name is not None:
            all_in.append(partition_name)
        from concourse.bass2jax import _bass_exec_p, partition_id_tensor

        def _body(*args):
            operands = list(args)
            if partition_name is not None:
                operands.append(partition_id_tensor())
            outs = _bass_exec_p.bind(
                *operands, out_avals=tuple(out_avals),
                in_names=tuple(all_in), out_names=tuple(out_names),
                lowering_input_output_aliases=(),
                sim_require_finite=False, sim_require_nnan=False, nc=nc)
            return tuple(outs)

        devices = jax.devices()[:n_cores]
        self.n_cores = n_cores
        self.mesh = Mesh(np.asarray(devices), ("core",))
        in_specs = (PartitionSpec("core"),) * (n_params + n_outs)
        out_specs = (PartitionSpec("core"),) * len(out_names)
        self.fn = jax.jit(
            shard_map(_body, mesh=self.mesh, in_specs=in_specs,
                      out_specs=out_specs, check_rep=False),
            keep_unused=True)
        self.dev_in = None

    def put(self, in_maps):
        from jax.sharding import NamedSharding, PartitionSpec
        sh = NamedSharding(self.mesh, PartitionSpec("core"))
        n = self.n_cores
        concat_in = [
            np.concatenate([np.asarray(in_maps[c][name]) for c in range(n)],
                           axis=0)
            for name in self.in_names]
        concat_zeros = [np.zeros((n * z.shape[0], *z.shape[1:]), z.dtype)
                        for z in self.zero_outs]
        self.dev_in = [self.jax.device_put(a, sh)
                       for a in concat_in + concat_zeros]
        self.jax.block_until_ready(self.dev_in)

    def __call__(self, fetch=("out",)):
        out = self.fn(*self.dev_in)
        self.jax.block_until_ready(out)
        n = self.n_cores
        return [
            {name: np.asarray(out[i]).reshape(n, *self.out_avals[i].shape)[c]
             for i, name in enumerate(self.out_names) if name in fetch}
            for c in range(n)]


_CACHE = {}


def _get_runner(N, D, groups, NT, n_own, n_pad, n_chunks):
    key = (N, D, NT)
    if key in _CACHE:
        return _CACHE[key]
    n_real_last = n_own - (n_chunks - 1) * P
    nc = _build(D, n_pad, n_chunks, NT, groups, n_real_last, N)
    r = _Runner(nc, CORES)
    _CACHE[key] = r
    return r


def _make_in_maps(prep, Ws_flat, gb):
    (xfull, idx_arr, rel_arr, norm_arr, groups, NT, n_own, n_pad,
     n_chunks) = prep
    return [{"xfull_in": xfull, "idx_in": idx_arr[c], "rel_in": rel_arr[c],
             "norm_in": norm_arr[c], "Ws_in": Ws_flat, "gb_in": gb}
            for c in range(CORES)]


def kernel(x, edge_index, Ws, bs, gammas, betas):
    x = np.asarray(x, np.float32)
    edge_index = np.asarray(edge_index, np.int32)
    Ws = np.asarray(Ws, np.float32)
    gammas = np.asarray(gammas, np.float32)
    betas = np.asarray(betas, np.float32)
    N, D = x.shape

    prep = _prep(x, edge_index)
    (xfull, idx_arr, rel_arr, norm_arr, groups, NT, n_own, n_pad,
     n_chunks) = prep
    r = _get_runner(N, D, groups, NT, n_own, n_pad, n_chunks)

    Ws_flat = Ws.reshape(L * D, D)
    gb = np.zeros((D, 2 * L), np.float32)
    for i in range(L):
        gb[:, 2 * i] = gammas[i]
        gb[:, 2 * i + 1] = betas[i]

    r.put(_make_in_maps(prep, Ws_flat, gb))
    res = r(fetch=("out", "outz"))
    x2 = np.empty((N, D), np.float32)
    z2 = np.empty((N, D), np.float32)
    for c in range(CORES):
        lo, hi = c * n_own, min((c + 1) * n_own, N)
        x2[lo:hi] = res[c]["out"][:, :hi - lo].T
        z2[lo:hi] = res[c]["outz"][:, :hi - lo].T
    # final BN (training-mode batch stats) + residual, exact in f64
    zf = z2.astype(np.float64)
    mean = zf.mean(axis=0)
    var = np.square(zf - mean).mean(axis=0)
    y2 = gammas[L - 1] * (z2 - mean) / np.sqrt(var + EPS) + betas[L - 1]
    return x2 + y2.astype(np.float32)
